# revision 32
# baseline (speedup 1.0000x reference)
"""CRF negative log-likelihood on 8 Trainium2 NeuronCores.

Strategy
--------
Data-parallel over batch (16 sequences per core), chunk-parallel over time
within each core. The forward recursion in the exp domain is

    u_t = exp(e_t - mu) * (M^T u_{t-1}),   M = exp(transitions)

M is a strongly mixing positive matrix (entries within 10% of 1), so a
1-step window product is numerically rank-1 and chunk chains can be
seeded with ones and stitched after the fact (rank-1 cross
approximation):

  logZ = log(end^T f_last) + sum_g [ log(1^T W_g f_{g-1}) - log(1^T W_g z) ]
         + T*mu

Schedule: emissions ship host-exp'd bf16 (DMA has headroom; Act doesn't).
Steps 1..527 form the D block: 31 chunks x 17 steps, advancing every
round (18 rounds): PE matmul -> DVE multiply straight from PSUM (1x).
Steps 528..1023 form 62 chunks x 8 steps in TWO COHORTS (X = even
chunks, Y = odd) that alternate rounds: cohort c matmuls on round r,
Act copies its PSUM to SBUF bf16 on round r+1, and the DVE multiplies it
there in 2x packed-bf16 mode. The staging chain (matmul -> copy ->
multiply) thus gets a two-round budget and stays off the critical path,
while per-round engine load is balanced:
DVE ~1000ns (496 cols 1x + 496 cols 2x), Act ~700ns (one 496-col copy),
PE ~414ns + filler matmuls that keep it continuously busy so it holds
its ramped 2.4 GHz p-state (it drops to 1.2 GHz when idling between
matmuls).

The Pool engine cannot read PSUM and its software tensor ops are ~2-4x
slower than DVE, so it only issues the streaming ee DMAs (SWDGE).

The boundary dot families come out as transposed column-sum matmuls and
are DMA'd out RAW; logs, subtraction and the per-sequence fold happen on
the host in float64. The gold-path score is host-gathered and summed on
the host (pure indexing of inputs).
"""

import json

import ml_dtypes
import numpy as np

import concourse.bass as bass
import concourse.tile as tile
import concourse.mybir as mybir
from concourse.bass_utils import run_bass_kernel_spmd
from concourse.vector_clock import ScopedClock

B, T, L = 128, 1024, 128
NCORES = 8
BL = B // NCORES          # 16 sequences per core
BOS, EOS = 126, 127
MU = float(np.log(126.0) + 0.5)

R = 18                    # global rounds
WD = 23 * BL              # D block: 15x17 + 8x16 step chunks, cols 0:368
WS = 40 * BL              # staged cols per cohort (40 chunks x 8 steps)
WTOT = WD + WS            # 1008 packed ee columns per round
NB = 102                  # chunk boundaries (22 D-internal + 80 staged)
NFLAT = NB * BL           # 1632 (boundary, seq) pairs
D1C, D2C = 15, 14         # psD1 / psD2 dot columns
D_OFFS = [1 + 17 * k for k in range(15)] + [256 + 16 * k for k in range(8)]
D_LENS = [17] * 15 + [16] * 8

FILL_PRE = 6              # prologue fillers of [L, 512]
FILL_LOOP = [512, 256]    # filler widths per round (p-state / DVFS pinning)

# ee DMA granules (round ranges); g0 on sync, the rest on gpsimd SWDGE
GRAN = [(0, 1), (1, 2), (2, 3), (3, 5), (5, 8), (8, 11), (11, 14), (14, 18)]

F32 = mybir.dt.float32
BF16 = mybir.dt.bfloat16
AF = mybir.ActivationFunctionType

TRACE = False             # set by test.py to capture an NTFF profile
LAST_RESULTS = None


# --------------------------------------------------------------------------
# Workaround for this walrus build: a Drain may carry at most ONE sync wait.
# Tile's tail drain waits on every outstanding DMA sem lane; split the waits
# across a chain of single-wait drains.
def _patch_tile_drain():
    if getattr(tile.TileContext, "_crf_drain_patched", False):
        return

    def _drain_and_barrier_split(self, tick_clock, wait_clock):
        nc = self.nc
        drain_inst = nc.sync.drain()
        wait_clock.add_sem_waits(
            drain_inst.ins, ScopedClock({None: tick_clock.global_clock})
        )
        si = drain_inst.ins.sync_info
        if si is not None and len(si.on_wait) > 1:
            waits = list(si.on_wait)
            drain_inst.ins.sync_info = mybir.SyncInfo(
                on_wait=[waits[0]], on_update=list(si.on_update)
            )
            for w in waits[1:]:
                d2 = nc.sync.drain()
                d2.ins.sync_info = mybir.SyncInfo(on_wait=[w], on_update=[])
        nc.all_engine_barrier()
        assert self.sems is not None
        popped = nc._tile_sem_poison_stack.pop()
        assert popped is self._sem_poison
        # The sem-clear ceremony (~6us of serial EVENT_SEMAPHORE traffic +
        # a second barrier) is skipped: the NEFF runs once per load and the
        # runtime reinitializes semaphore state on each execution.
        nc.free_semaphores_without_clearing(
            list(self.sems.allocated().values())
        ) if hasattr(nc, "free_semaphores_without_clearing") else None

    tile.TileContext._drain_and_barrier = _drain_and_barrier_split
    tile.TileContext._crf_drain_patched = True


# This walrus build rejects instructions carrying more than one sync wait
# ("Too many sync wait commands"). Post-process the serialized BIR: move
# excess waits onto NoOp instructions inserted just before the owner.
_MAX_WAITS = 1
_PRELOAD_WAITS = []      # (inst_name, sem_name, value) injected post-Tile
_PRELOAD_DMAS = []       # DMA inst names whose sem update must be add-imm


def _split_sync_waits_json(raw: bytes) -> bytes:
    m = json.loads(raw)
    nid = [0]
    pre = {n: (s, v) for n, s, v in _PRELOAD_WAITS}
    sem_ids = {}
    for f in m.get("functions", []):
        for s in f.get("semaphores", []):
            sem_ids[s.get("ant_name", s.get("name"))] = s.get("id")
    for f in m.get("functions", []):
        for bb in f.get("blocks", []):
            out = []
            for ins in bb.get("instructions", []):
                if ins.get("name") in _PRELOAD_DMAS:
                    for u in (ins.get("sync_info") or {}).get("on_update", []):
                        if u.get("ant_name") == "preload":
                            u["update_mode"] = "sem-add-imm"
                            u["update_value"] = 16
                if ins.get("name") in pre:
                    sname, val = pre[ins.get("name")]
                    nid[0] += 1
                    out.append({
                        "engine": ins["engine"], "ins": [],
                        "name": f"I-prewait-{nid[0]}", "opcode": "NoOp",
                        "outs": [],
                        "sync_info": {"on_update": [], "on_wait": [{
                            "ant_name": sname, "id": sem_ids.get(sname, 0),
                            "sync_type": "semaphore",
                            "wait_mode": "sem-ge-imm", "wait_value": val}]},
                    })
                si = ins.get("sync_info")
                waits = (si or {}).get("on_wait") or []
                if len(waits) > _MAX_WAITS:
                    # Keep the most-likely-critical wait on the real
                    # instruction (cross-engine compute producer, PE first);
                    # stale waits (same-engine slot reuse, DMA long done) go
                    # to the NoOps so they retire early.
                    eng = ins.get("engine", "")
                    prio = {"PE": 4, "Pool": 3, "Activation": 2}

                    def _score(w):
                        p = w.get("ant_name", "").split("_")[0]
                        if p == eng:
                            return 0
                        if p.startswith("DMA"):
                            return 1
                        return prio.get(p, 2)

                    # Same-engine sem waits are trivially satisfied on an
                    # in-order engine (no Tile loops -> no sem resets): drop.
                    waits = [
                        w
                        for w in waits
                        if w.get("ant_name", "").split("_")[0] != eng
                    ] or waits[-1:]
                    waits = sorted(waits, key=_score)
                    extra, keep = waits[:-_MAX_WAITS], waits[-_MAX_WAITS:]
                    for w in extra:
                        nid[0] += 1
                        out.append(
                            {
                                "engine": ins["engine"],
                                "ins": [],
                                "name": f"I-waitsplit-{nid[0]}",
                                "opcode": "NoOp",
                                "outs": [],
                                "sync_info": {"on_update": [], "on_wait": [w]},
                            }
                        )
                    si["on_wait"] = keep
                out.append(ins)
            bb["instructions"] = out
    return json.dumps(m).encode()


def _patch_to_json():
    if getattr(bass.Bass, "_crf_json_patched", False):
        return
    orig = bass.Bass.to_json_bytes

    def to_json_split(self, *a, **kw):
        return _split_sync_waits_json(orig(self, *a, **kw))

    bass.Bass.to_json_bytes = to_json_split
    bass.Bass._crf_json_patched = True


# --------------------------------------------------------------------------
def build_bass():
    _patch_tile_drain()
    _patch_to_json()

    nc = bass.Bass("TRN2")
    ee_d = nc.dram_tensor("ee", [L, R, WTOT], BF16, kind="ExternalInput")
    u0_d = nc.dram_tensor("u0", [L, BL], BF16, kind="ExternalInput")
    expa_d = nc.dram_tensor("expa", [L, L], BF16, kind="ExternalInput")
    dots_d = nc.dram_tensor("dots_out", [L, D1C + D2C + 1], F32,
                            kind="ExternalOutput")

    with tile.TileContext(nc) as tc:
        with (
            tc.tile_pool(name="consts", bufs=1) as consts,
            tc.tile_pool(name="ud", bufs=3) as ud_pool,
            tc.tile_pool(name="ux", bufs=2) as ux_pool,
            tc.tile_pool(name="uy", bufs=2) as uy_pool,
            tc.tile_pool(name="sg", bufs=2) as sg_pool,
            tc.tile_pool(name="psd", bufs=2, space="PSUM") as psd_pool,
            tc.tile_pool(name="psx", bufs=1, space="PSUM") as psx_pool,
            tc.tile_pool(name="psy", bufs=1, space="PSUM") as psy_pool,
            tc.tile_pool(name="ps_fill", bufs=1, space="PSUM") as ps_fill,
            tc.tile_pool(name="ps_dots", bufs=1, space="PSUM") as ps_dots,
        ):
            # ---- PE warm-up fodder: no DMA dependencies ------------------
            dummy = consts.tile([L, 512], BF16)
            nc.vector.memset(dummy, 1.0)
            fillp = ps_fill.tile([L, 512], F32, name="fill", tag="fill")

            def filler(w=512):
                nc.tensor.matmul(fillp[:, 0:w], dummy[:, 0:L], dummy[:, 0:w],
                                 skip_group_check=True)

            for _ in range(FILL_PRE):
                filler()

            # ---- DMAs ----------------------------------------------------
            # expa/u0 on the Act queue, ee granule 0 on sync, the ee
            # stream on gpsimd SWDGE: three queues in parallel.
            ee_t = []
            for gi, (lo, hi) in enumerate(GRAN):
                ee_t.append((lo, hi, consts.tile(
                    [L, (hi - lo) * WTOT], BF16, name=f"ee{gi}",
                    tag=f"ee{gi}")))
            expa_sb = consts.tile([L, L], BF16)
            uD = ud_pool.tile([L, WD], BF16, tag="uD")
            nc.vector.memset(uD[:, BL:WD], 1.0)
            uX = ux_pool.tile([L, WS], BF16, tag="uX")
            nc.vector.memset(uX, 1.0)
            uY = uy_pool.tile([L, WS], BF16, tag="uY")
            nc.vector.memset(uY, 1.0)

            nc.scalar.dma_start(out=expa_sb, in_=expa_d[:, :])
            nc.scalar.dma_start(out=uD[:, 0:BL], in_=u0_d[:, :])
            nc.sync.dma_start(out=ee_t[0][2], in_=ee_d[:, 0:1, :])
            # gate the bulk ee stream on expa's arrival (read-dep dummy) so
            # the round-0 critical transfers aren't fighting it for the bus
            nc.gpsimd.tensor_copy(out=ee_t[1][2][:, 0:2],
                                  in_=expa_sb[:, 0:2])
            for gi, (lo, hi) in enumerate(GRAN):
                if gi > 0:
                    nc.gpsimd.dma_start(out=ee_t[gi][2], in_=ee_d[:, lo:hi, :])

            def ee2(r, c0, c1):
                for lo, hi, et in ee_t:
                    if lo <= r < hi:
                        base = (r - lo) * WTOT
                        return et[:, base + c0 : base + c1]
                raise AssertionError(r)

            ones_b16 = consts.tile([L, 1], BF16)
            nc.vector.memset(ones_b16, 1.0)
            endcol = consts.tile([L, 1], F32)
            nc.scalar.activation(out=endcol, in_=expa_sb[:, EOS : EOS + 1],
                                 func=AF.Copy)

            # transposed blocked column-sums: dots spread across
            # partitions; every matmul writes at partition 0 (HW rule), so
            # each segment starts a fresh psum column
            def blocked_colsums(psD, segs, col=0):
                for tile_, lo, hi in segs:
                    pos = lo
                    while pos < hi:
                        n = min(hi - pos, L)
                        nc.tensor.matmul(
                            psD[0:n, col : col + 1],
                            tile_[:, pos : pos + n],
                            ones_b16,
                            skip_group_check=True,
                        )
                        pos += n
                        col += 1

            # single PSUM bank: cols 0:D1C numerators (+ end term), the
            # rest denominators
            psDD = ps_dots.tile([L, D1C + D2C], F32, name="dd", tag="dd")
            psD1 = psDD[:, 0:D1C]
            psD2 = psDD[:, D1C : D1C + D2C]
            fe = consts.tile([L, BL], BF16)
            dots_sb = consts.tile([L, D1C + D2C + 1], F32)

            # ---- main scan -----------------------------------------------
            # D (23 chunks, 17/16 steps): matmul+multiply every round.
            # X (staged even chunks): matmul on even rounds, staged multiply
            # one round later; Y (odd chunks) on the odd-round phase.
            psX = psY = None
            for r in range(R):
                psD = psd_pool.tile([L, 512], F32, tag="psD")
                nc.tensor.matmul(psD[:, 0:WD], expa_sb, uD)
                # cohort matmul: 640 cols split at the PSUM bank boundary
                if r % 2 == 0:
                    psX = psx_pool.tile([L, 1024], F32, tag="psX")
                    nc.tensor.matmul(psX[:, 0:512], expa_sb, uX[:, 0:512])
                    nc.tensor.matmul(psX[:, 512:WS], expa_sb, uX[:, 512:WS])
                else:
                    psY = psy_pool.tile([L, 1024], F32, tag="psY")
                    nc.tensor.matmul(psY[:, 0:512], expa_sb, uY[:, 0:512])
                    nc.tensor.matmul(psY[:, 512:WS], expa_sb, uY[:, 512:WS])
                for w in FILL_LOOP:
                    filler(w)

                uD = ud_pool.tile([L, WD], BF16, tag="uD")
                nc.vector.tensor_mul(uD, psD[:, 0:WD], ee2(r, 0, WD))

                if r >= 1:
                    prev_is_x = (r - 1) % 2 == 0
                    pprev = psX if prev_is_x else psY
                    sg = sg_pool.tile([L, WS], BF16, tag="sg")
                    nc.scalar.activation(out=sg, in_=pprev[:, 0:WS],
                                         func=AF.Copy)
                    if prev_is_x:
                        uX = ux_pool.tile([L, WS], BF16, tag="uX")
                        nc.vector.tensor_mul(uX, sg, ee2(r - 1, WD, WTOT))
                    else:
                        uY = uy_pool.tile([L, WS], BF16, tag="uY")
                        nc.vector.tensor_mul(uY, sg, ee2(r - 1, WD, WTOT))

                if r == 0:
                    # D-internal denominators: 1^T W_k z for chunks 1..22
                    blocked_colsums(psD2, [(uD, BL, WD)], col=0)
                if r == 1:
                    # X chunks' first-step states: denominators for the
                    # D22->X0 boundary (col 3) and the Y->X boundaries
                    # (cols 9:14)
                    blocked_colsums(psD2, [(uX, 0, BL)], col=3)
                    blocked_colsums(psD2, [(uX, BL, WS)], col=9)
                if r == 2:
                    # Y chunks' first-step states: X->Y denominators
                    blocked_colsums(psD2, [(uY, 0, WS)], col=4)
                if r == 4:
                    nc.scalar.activation(out=dots_sb[:, D1C : D1C + D2C],
                                         in_=psD2, func=AF.Copy)
                if r == 16:
                    # 16-step D chunks reached their extension this round;
                    # round 17 would overwrite them with padding
                    blocked_colsums(psD1, [(uD, 240, 352)], col=2)
                    blocked_colsums(psD1, [(uD, 352, WD)], col=3)
                    # end term: chunk 79 (Y) finished its 8 own steps at the
                    # round-16 staged multiply
                    nc.vector.tensor_scalar_mul(
                        out=fe, in0=uY[:, WS - BL : WS], scalar1=endcol)
                if r == 17:
                    nc.tensor.matmul(psD1[0:BL, D1C - 1 : D1C], fe,
                                     ones_b16, skip_group_check=True)

            # ---- numerators ----------------------------------------------
            # X extension states landed with the round-17 staged multiply
            blocked_colsums(psD1, [(uX, 0, WS)], col=4)
            # Y extension: psY holds the round-17 matmul; stage + multiply
            sg = sg_pool.tile([L, WS], BF16, tag="sg")
            nc.scalar.activation(out=sg, in_=psY[:, 0:WS], func=AF.Copy)
            blocked_colsums(psD1, [(uD, 0, 240)], col=0)
            uY = uy_pool.tile([L, WS], BF16, tag="uY")
            nc.vector.tensor_mul(uY, sg, ee2(R - 1, WD, WTOT))
            blocked_colsums(psD1, [(uY, 0, WS - BL)], col=9)
            nc.scalar.activation(out=dots_sb[:, 0:D1C], in_=psD1,
                                 func=AF.Copy)
            nc.sync.dma_start(out=dots_d[:, :], in_=dots_sb)

    return nc


# --------------------------------------------------------------------------
def _host_prep(emissions, tags, mask, transitions):
    em = np.asarray(emissions, dtype=np.float32)
    tr = np.asarray(transitions, dtype=np.float32)
    expa = np.exp(tr.astype(np.float64)).astype(ml_dtypes.bfloat16)

    in_maps = []
    for core in range(NCORES):
        s = slice(core * BL, (core + 1) * BL)
        emC = em[s]                                         # (BL, T, L)
        emT = np.ascontiguousarray(emC.transpose(2, 1, 0))  # (L, T, BL)

        ee = np.zeros((L, R, WTOT), np.float32)
        # D block: 23 chunks (15x17 + 8x16 steps + 1 ext round each); each
        # ext row is the successor chunk's first step (chunk 22 rolls into
        # the staged block at 384).
        for k in range(23):
            off, n = D_OFFS[k], D_LENS[k] + 1
            ee[:, 0:n, BL * k : BL * (k + 1)] = emT[:, off : off + n, :]
        # staged: 80 chunks x 8 steps, off 384+8j; X = even j on even
        # rounds (step s at round 2s), Y = odd j one round later. The ext
        # step s=8 is the successor chunk's first step; chunk 79 has no
        # successor (its ext stays 0, never read).
        for j in range(80):
            off = 384 + 8 * j
            c0 = WD + BL * (j // 2)
            par = j % 2
            for s in range(9):
                t, r = off + s, 2 * s + par
                if t < T and r < R:
                    ee[:, r, c0 : c0 + BL] = emT[:, t, :]
        ee = np.exp(ee - MU).astype(ml_dtypes.bfloat16)

        u0 = np.exp(emT[:, 0, :] + tr[BOS, :][:, None] - MU)
        in_maps.append(
            {
                "ee": ee,
                "u0": u0.astype(ml_dtypes.bfloat16),
                "expa": expa,
            }
        )
    return in_maps


def _host_scores(emissions, tags, mask, transitions):
    """Gold-path score: pure index gathers + sum, in float64."""
    em = np.asarray(emissions, dtype=np.float32)
    tg = np.asarray(tags).astype(np.int64)
    mk = np.asarray(mask, dtype=np.float32)
    tr = np.asarray(transitions, dtype=np.float32)

    eg = np.take_along_axis(em, tg[:, :, None], axis=2)[:, :, 0]    # (B,T)
    eg = eg * np.concatenate([np.ones((B, 1), np.float32), mk[:, 1:]], 1)
    tp = tr[tg[:, :-1], tg[:, 1:]] * mk[:, 1:]                      # (B,T-1)
    bos_t = tr[BOS, tg[:, 0]]
    last_idx = mk.astype(np.int64).sum(axis=1) - 1
    last_tags = np.take_along_axis(tg, last_idx[:, None], axis=1)[:, 0]
    eos_t = tr[last_tags, EOS]
    return (eg.astype(np.float64).sum(1) + tp.astype(np.float64).sum(1)
            + bos_t + eos_t)                                        # (B,)


# flat layouts: (column, rows) blocks aligning numerator/denominator pairs
# per boundary; see build_bass for the segment order.
_BLOCKS1 = [(0, 128), (1, 112), (2, 112), (3, 16),
            (4, 128), (5, 128), (6, 128), (7, 128), (8, 128),
            (9, 128), (10, 128), (11, 128), (12, 128), (13, 112)]
_BLOCKS2 = [(0, 128), (1, 128), (2, 96), (3, 16),
            (4, 128), (5, 128), (6, 128), (7, 128), (8, 128),
            (9, 128), (10, 128), (11, 128), (12, 128), (13, 112)]


def _host_logz(dots):
    """Fold one core's raw dot tensor into per-seq logZ (16,)."""
    d = dots.astype(np.float64)
    d1, d2 = d[:, 0:D1C], d[:, D1C : D1C + D2C]
    n1 = np.concatenate([d1[0:n, c] for c, n in _BLOCKS1])   # (NFLAT,)
    n2 = np.concatenate([d2[0:n, c] for c, n in _BLOCKS2])
    bnd = np.log(n1) - np.log(n2)
    logz = bnd.reshape(NB, BL).sum(axis=0)
    logz += np.log(d1[0:BL, D1C - 1])                        # end term
    return logz + float(T) * MU


_NC_CACHE = {}


def kernel(emissions, tags, mask, transitions):
    global LAST_RESULTS
    if "nc" not in _NC_CACHE:
        _NC_CACHE["nc"] = build_bass()
    nc = _NC_CACHE["nc"]
    in_maps = _host_prep(emissions, tags, mask, transitions)
    res = run_bass_kernel_spmd(
        nc, in_maps, core_ids=list(range(NCORES)), trace=TRACE
    )
    LAST_RESULTS = res
    scores = _host_scores(emissions, tags, mask, transitions)
    logz = np.concatenate([_host_logz(r["dots_out"]) for r in res.results])
    return np.float32(-(scores - logz).mean())


# revision 33
# speedup vs baseline: 1.1209x; 1.1209x over previous
"""CRF negative log-likelihood on 8 Trainium2 NeuronCores.

Strategy
--------
Data-parallel over batch (16 sequences per core), chunk-parallel over time
within each core. The forward recursion in the exp domain is

    u_t = exp(e_t - mu) * (M^T u_{t-1}),   M = exp(transitions)

M is a strongly mixing positive matrix (entries within 10% of 1), so a
1-step window product is numerically rank-1 and chunk chains can be
seeded with ones and stitched after the fact (rank-1 cross
approximation):

  logZ = log(end^T f_last) + sum_g [ log(1^T W_g f_{g-1}) - log(1^T W_g z) ]
         + T*mu

Schedule: emissions ship host-exp'd bf16 (DMA has headroom; Act doesn't).
Steps 1..527 form the D block: 31 chunks x 17 steps, advancing every
round (18 rounds): PE matmul -> DVE multiply straight from PSUM (1x).
Steps 528..1023 form 62 chunks x 8 steps in TWO COHORTS (X = even
chunks, Y = odd) that alternate rounds: cohort c matmuls on round r,
Act copies its PSUM to SBUF bf16 on round r+1, and the DVE multiplies it
there in 2x packed-bf16 mode. The staging chain (matmul -> copy ->
multiply) thus gets a two-round budget and stays off the critical path,
while per-round engine load is balanced:
DVE ~1000ns (496 cols 1x + 496 cols 2x), Act ~700ns (one 496-col copy),
PE ~414ns + filler matmuls that keep it continuously busy so it holds
its ramped 2.4 GHz p-state (it drops to 1.2 GHz when idling between
matmuls).

The Pool engine cannot read PSUM and its software tensor ops are ~2-4x
slower than DVE, so it only issues the streaming ee DMAs (SWDGE).

The boundary dot families come out as transposed column-sum matmuls and
are DMA'd out RAW; logs, subtraction and the per-sequence fold happen on
the host in float64. The gold-path score is host-gathered and summed on
the host (pure indexing of inputs).
"""

import json

import ml_dtypes
import numpy as np

import concourse.bass as bass
import concourse.tile as tile
import concourse.mybir as mybir
from concourse.bass_utils import run_bass_kernel_spmd
from concourse.vector_clock import ScopedClock

B, T, L = 128, 1024, 128
NCORES = 8
BL = B // NCORES          # 16 sequences per core
BOS, EOS = 126, 127
MU = float(np.log(126.0) + 0.5)

R = 18                    # global rounds
WD = 23 * BL              # D block: 15x17 + 8x16 step chunks, cols 0:368
WS = 40 * BL              # staged cols per cohort (40 chunks x 8 steps)
WTOT = WD + WS            # 1008 packed ee columns per round
NB = 102                  # chunk boundaries (22 D-internal + 80 staged)
NFLAT = NB * BL           # 1632 (boundary, seq) pairs
D1C, D2C = 15, 14         # psD1 / psD2 dot columns
D_OFFS = [1 + 17 * k for k in range(15)] + [256 + 16 * k for k in range(8)]
D_LENS = [17] * 15 + [16] * 8

FILL_PRE = 6              # prologue fillers of [L, 512]
FILL_LOOP = [512, 256]    # filler widths per round (p-state / DVFS pinning)

# ee DMA granules (round ranges); g0 on sync, the rest on gpsimd SWDGE
GRAN = [(0, 1), (1, 2), (2, 3), (3, 5), (5, 8), (8, 11), (11, 14), (14, 18)]

F32 = mybir.dt.float32
BF16 = mybir.dt.bfloat16
AF = mybir.ActivationFunctionType

TRACE = False             # set by test.py to capture an NTFF profile
LAST_RESULTS = None


# --------------------------------------------------------------------------
# Workaround for this walrus build: a Drain may carry at most ONE sync wait.
# Tile's tail drain waits on every outstanding DMA sem lane; split the waits
# across a chain of single-wait drains.
def _patch_tile_drain():
    if getattr(tile.TileContext, "_crf_drain_patched", False):
        return

    def _drain_and_barrier_split(self, tick_clock, wait_clock):
        nc = self.nc
        drain_inst = nc.sync.drain()
        wait_clock.add_sem_waits(
            drain_inst.ins, ScopedClock({None: tick_clock.global_clock})
        )
        si = drain_inst.ins.sync_info
        if si is not None and len(si.on_wait) > 1:
            waits = list(si.on_wait)
            drain_inst.ins.sync_info = mybir.SyncInfo(
                on_wait=[waits[0]], on_update=list(si.on_update)
            )
            for w in waits[1:]:
                d2 = nc.sync.drain()
                d2.ins.sync_info = mybir.SyncInfo(on_wait=[w], on_update=[])
        nc.all_engine_barrier()
        assert self.sems is not None
        popped = nc._tile_sem_poison_stack.pop()
        assert popped is self._sem_poison
        # The sem-clear ceremony (~6us of serial EVENT_SEMAPHORE traffic +
        # a second barrier) is skipped: the NEFF runs once per load and the
        # runtime reinitializes semaphore state on each execution.
        nc.free_semaphores_without_clearing(
            list(self.sems.allocated().values())
        ) if hasattr(nc, "free_semaphores_without_clearing") else None

    tile.TileContext._drain_and_barrier = _drain_and_barrier_split
    tile.TileContext._crf_drain_patched = True


# This walrus build rejects instructions carrying more than one sync wait
# ("Too many sync wait commands"). Post-process the serialized BIR: move
# excess waits onto NoOp instructions inserted just before the owner.
_MAX_WAITS = 1
_PRELOAD_WAITS = []      # (inst_name, sem_name, value) injected post-Tile
_PRELOAD_DMAS = []       # DMA inst names whose sem update must be add-imm


def _split_sync_waits_json(raw: bytes) -> bytes:
    m = json.loads(raw)
    nid = [0]
    pre = {n: (s, v) for n, s, v in _PRELOAD_WAITS}
    sem_ids = {}
    for f in m.get("functions", []):
        for s in f.get("semaphores", []):
            sem_ids[s.get("ant_name", s.get("name"))] = s.get("id")
    for f in m.get("functions", []):
        for bb in f.get("blocks", []):
            out = []
            for ins in bb.get("instructions", []):
                if ins.get("name") in _PRELOAD_DMAS:
                    for u in (ins.get("sync_info") or {}).get("on_update", []):
                        if u.get("ant_name") == "preload":
                            u["update_mode"] = "sem-add-imm"
                            u["update_value"] = 16
                if ins.get("name") in pre:
                    sname, val = pre[ins.get("name")]
                    nid[0] += 1
                    out.append({
                        "engine": ins["engine"], "ins": [],
                        "name": f"I-prewait-{nid[0]}", "opcode": "NoOp",
                        "outs": [],
                        "sync_info": {"on_update": [], "on_wait": [{
                            "ant_name": sname, "id": sem_ids.get(sname, 0),
                            "sync_type": "semaphore",
                            "wait_mode": "sem-ge-imm", "wait_value": val}]},
                    })
                si = ins.get("sync_info")
                waits = (si or {}).get("on_wait") or []
                if len(waits) > _MAX_WAITS:
                    # Keep the most-likely-critical wait on the real
                    # instruction (cross-engine compute producer, PE first);
                    # stale waits (same-engine slot reuse, DMA long done) go
                    # to the NoOps so they retire early.
                    eng = ins.get("engine", "")
                    prio = {"PE": 4, "Pool": 3, "Activation": 2}

                    def _score(w):
                        p = w.get("ant_name", "").split("_")[0]
                        if p == eng:
                            return 0
                        if p.startswith("DMA"):
                            return 1
                        return prio.get(p, 2)

                    # Same-engine sem waits are trivially satisfied on an
                    # in-order engine (no Tile loops -> no sem resets): drop.
                    waits = [
                        w
                        for w in waits
                        if w.get("ant_name", "").split("_")[0] != eng
                    ] or waits[-1:]
                    waits = sorted(waits, key=_score)
                    extra, keep = waits[:-_MAX_WAITS], waits[-_MAX_WAITS:]
                    for w in extra:
                        nid[0] += 1
                        out.append(
                            {
                                "engine": ins["engine"],
                                "ins": [],
                                "name": f"I-waitsplit-{nid[0]}",
                                "opcode": "NoOp",
                                "outs": [],
                                "sync_info": {"on_update": [], "on_wait": [w]},
                            }
                        )
                    si["on_wait"] = keep
                out.append(ins)
            bb["instructions"] = out
    return json.dumps(m).encode()


def _patch_to_json():
    if getattr(bass.Bass, "_crf_json_patched", False):
        return
    orig = bass.Bass.to_json_bytes

    def to_json_split(self, *a, **kw):
        return _split_sync_waits_json(orig(self, *a, **kw))

    bass.Bass.to_json_bytes = to_json_split
    bass.Bass._crf_json_patched = True


# --------------------------------------------------------------------------
def build_bass():
    _patch_tile_drain()
    _patch_to_json()

    nc = bass.Bass("TRN2")
    ee_d = nc.dram_tensor("ee", [L, R, WTOT], BF16, kind="ExternalInput")
    u0_d = nc.dram_tensor("u0", [L, BL], BF16, kind="ExternalInput")
    expa_d = nc.dram_tensor("expa", [L, L], BF16, kind="ExternalInput")
    dots_d = nc.dram_tensor("dots_out", [L, D1C + D2C + 1], F32,
                            kind="ExternalOutput")

    with tile.TileContext(nc) as tc:
        with (
            tc.tile_pool(name="consts", bufs=1) as consts,
            tc.tile_pool(name="ud", bufs=3) as ud_pool,
            tc.tile_pool(name="ux", bufs=2) as ux_pool,
            tc.tile_pool(name="uy", bufs=2) as uy_pool,
            tc.tile_pool(name="sg", bufs=2) as sg_pool,
            tc.tile_pool(name="psd", bufs=2, space="PSUM") as psd_pool,
            tc.tile_pool(name="psx", bufs=1, space="PSUM") as psx_pool,
            tc.tile_pool(name="psy", bufs=1, space="PSUM") as psy_pool,
            tc.tile_pool(name="ps_fill", bufs=1, space="PSUM") as ps_fill,
            tc.tile_pool(name="ps_dots", bufs=1, space="PSUM") as ps_dots,
        ):
            # ---- PE warm-up fodder: no DMA dependencies ------------------
            dummy = consts.tile([L, 512], BF16)
            nc.vector.memset(dummy, 1.0)
            fillp = ps_fill.tile([L, 512], F32, name="fill", tag="fill")

            def filler(w=512):
                nc.tensor.matmul(fillp[:, 0:w], dummy[:, 0:L], dummy[:, 0:w],
                                 skip_group_check=True)

            for _ in range(FILL_PRE):
                filler()

            # ---- DMAs ----------------------------------------------------
            # expa/u0 on the Act queue, ee granule 0 on sync, the ee
            # stream on gpsimd SWDGE: three queues in parallel.
            ee_t = []
            for gi, (lo, hi) in enumerate(GRAN):
                ee_t.append((lo, hi, consts.tile(
                    [L, (hi - lo) * WTOT], BF16, name=f"ee{gi}",
                    tag=f"ee{gi}")))
            expa_sb = consts.tile([L, L], BF16)
            uD = ud_pool.tile([L, WD], BF16, tag="uD")
            nc.vector.memset(uD[:, BL:WD], 1.0)
            uX = ux_pool.tile([L, WS], BF16, tag="uX")
            nc.vector.memset(uX, 1.0)
            uY = uy_pool.tile([L, WS], BF16, tag="uY")
            nc.vector.memset(uY, 1.0)

            nc.scalar.dma_start(out=expa_sb, in_=expa_d[:, :])
            nc.scalar.dma_start(out=uD[:, 0:BL], in_=u0_d[:, :])
            nc.sync.dma_start(out=ee_t[0][2], in_=ee_d[:, 0:1, :])
            nc.sync.dma_start(out=ee_t[1][2], in_=ee_d[:, 1:2, :])
            for gi, (lo, hi) in enumerate(GRAN):
                if gi > 1:
                    nc.gpsimd.dma_start(out=ee_t[gi][2], in_=ee_d[:, lo:hi, :])

            def ee2(r, c0, c1):
                for lo, hi, et in ee_t:
                    if lo <= r < hi:
                        base = (r - lo) * WTOT
                        return et[:, base + c0 : base + c1]
                raise AssertionError(r)

            ones_b16 = consts.tile([L, 1], BF16)
            nc.vector.memset(ones_b16, 1.0)
            endcol = consts.tile([L, 1], F32)
            nc.scalar.activation(out=endcol, in_=expa_sb[:, EOS : EOS + 1],
                                 func=AF.Copy)

            # transposed blocked column-sums: dots spread across
            # partitions; every matmul writes at partition 0 (HW rule), so
            # each segment starts a fresh psum column
            def blocked_colsums(psD, segs, col=0):
                for tile_, lo, hi in segs:
                    pos = lo
                    while pos < hi:
                        n = min(hi - pos, L)
                        nc.tensor.matmul(
                            psD[0:n, col : col + 1],
                            tile_[:, pos : pos + n],
                            ones_b16,
                            skip_group_check=True,
                        )
                        pos += n
                        col += 1

            # single PSUM bank: cols 0:D1C numerators (+ end term), the
            # rest denominators
            psDD = ps_dots.tile([L, D1C + D2C], F32, name="dd", tag="dd")
            psD1 = psDD[:, 0:D1C]
            psD2 = psDD[:, D1C : D1C + D2C]
            fe = consts.tile([L, BL], BF16)
            dots_sb = consts.tile([L, D1C + D2C + 1], F32)

            # ---- main scan -----------------------------------------------
            # D (23 chunks, 17/16 steps): matmul+multiply every round.
            # X (staged even chunks): matmul on even rounds, staged multiply
            # one round later; Y (odd chunks) on the odd-round phase.
            psX = psY = None
            for r in range(R):
                psD = psd_pool.tile([L, 512], F32, tag="psD")
                nc.tensor.matmul(psD[:, 0:WD], expa_sb, uD)
                # cohort matmul: 640 cols split at the PSUM bank boundary
                if r % 2 == 0:
                    psX = psx_pool.tile([L, 1024], F32, tag="psX")
                    nc.tensor.matmul(psX[:, 0:512], expa_sb, uX[:, 0:512])
                    nc.tensor.matmul(psX[:, 512:WS], expa_sb, uX[:, 512:WS])
                else:
                    psY = psy_pool.tile([L, 1024], F32, tag="psY")
                    nc.tensor.matmul(psY[:, 0:512], expa_sb, uY[:, 0:512])
                    nc.tensor.matmul(psY[:, 512:WS], expa_sb, uY[:, 512:WS])
                for w in FILL_LOOP:
                    filler(w)

                uD = ud_pool.tile([L, WD], BF16, tag="uD")
                nc.vector.tensor_mul(uD, psD[:, 0:WD], ee2(r, 0, WD))

                if r >= 1:
                    prev_is_x = (r - 1) % 2 == 0
                    pprev = psX if prev_is_x else psY
                    sg = sg_pool.tile([L, WS], BF16, tag="sg")
                    nc.scalar.activation(out=sg, in_=pprev[:, 0:WS],
                                         func=AF.Copy)
                    if prev_is_x:
                        uX = ux_pool.tile([L, WS], BF16, tag="uX")
                        nc.vector.tensor_mul(uX, sg, ee2(r - 1, WD, WTOT))
                    else:
                        uY = uy_pool.tile([L, WS], BF16, tag="uY")
                        nc.vector.tensor_mul(uY, sg, ee2(r - 1, WD, WTOT))

                if r == 0:
                    # D-internal denominators: 1^T W_k z for chunks 1..22
                    blocked_colsums(psD2, [(uD, BL, WD)], col=0)
                if r == 1:
                    # X chunks' first-step states: denominators for the
                    # D22->X0 boundary (col 3) and the Y->X boundaries
                    # (cols 9:14)
                    blocked_colsums(psD2, [(uX, 0, BL)], col=3)
                    blocked_colsums(psD2, [(uX, BL, WS)], col=9)
                if r == 2:
                    # Y chunks' first-step states: X->Y denominators
                    blocked_colsums(psD2, [(uY, 0, WS)], col=4)
                if r == 4:
                    nc.scalar.activation(out=dots_sb[:, D1C : D1C + D2C],
                                         in_=psD2, func=AF.Copy)
                if r == 16:
                    # 16-step D chunks reached their extension this round;
                    # round 17 would overwrite them with padding
                    blocked_colsums(psD1, [(uD, 240, 352)], col=2)
                    blocked_colsums(psD1, [(uD, 352, WD)], col=3)
                    # end term: chunk 79 (Y) finished its 8 own steps at the
                    # round-16 staged multiply
                    nc.vector.tensor_scalar_mul(
                        out=fe, in0=uY[:, WS - BL : WS], scalar1=endcol)
                if r == 17:
                    nc.tensor.matmul(psD1[0:BL, D1C - 1 : D1C], fe,
                                     ones_b16, skip_group_check=True)

            # ---- numerators ----------------------------------------------
            # X extension states landed with the round-17 staged multiply
            blocked_colsums(psD1, [(uX, 0, WS)], col=4)
            # Y extension: psY holds the round-17 matmul; multiply it
            # directly (1x) so the tail skips the Act staging hop
            blocked_colsums(psD1, [(uD, 0, 240)], col=0)
            uY = uy_pool.tile([L, WS], BF16, tag="uY")
            nc.vector.tensor_mul(uY, psY[:, 0:WS], ee2(R - 1, WD, WTOT))
            blocked_colsums(psD1, [(uY, 0, WS - BL)], col=9)
            nc.scalar.activation(out=dots_sb[:, 0:D1C], in_=psD1,
                                 func=AF.Copy)
            nc.sync.dma_start(out=dots_d[:, :], in_=dots_sb)

    return nc


# --------------------------------------------------------------------------
def _host_prep(emissions, tags, mask, transitions):
    em = np.asarray(emissions, dtype=np.float32)
    tr = np.asarray(transitions, dtype=np.float32)
    expa = np.exp(tr.astype(np.float64)).astype(ml_dtypes.bfloat16)

    in_maps = []
    for core in range(NCORES):
        s = slice(core * BL, (core + 1) * BL)
        emC = em[s]                                         # (BL, T, L)
        emT = np.ascontiguousarray(emC.transpose(2, 1, 0))  # (L, T, BL)

        ee = np.zeros((L, R, WTOT), np.float32)
        # D block: 23 chunks (15x17 + 8x16 steps + 1 ext round each); each
        # ext row is the successor chunk's first step (chunk 22 rolls into
        # the staged block at 384).
        for k in range(23):
            off, n = D_OFFS[k], D_LENS[k] + 1
            ee[:, 0:n, BL * k : BL * (k + 1)] = emT[:, off : off + n, :]
        # staged: 80 chunks x 8 steps, off 384+8j; X = even j on even
        # rounds (step s at round 2s), Y = odd j one round later. The ext
        # step s=8 is the successor chunk's first step; chunk 79 has no
        # successor (its ext stays 0, never read).
        for j in range(80):
            off = 384 + 8 * j
            c0 = WD + BL * (j // 2)
            par = j % 2
            for s in range(9):
                t, r = off + s, 2 * s + par
                if t < T and r < R:
                    ee[:, r, c0 : c0 + BL] = emT[:, t, :]
        ee = np.exp(ee - MU).astype(ml_dtypes.bfloat16)

        u0 = np.exp(emT[:, 0, :] + tr[BOS, :][:, None] - MU)
        in_maps.append(
            {
                "ee": ee,
                "u0": u0.astype(ml_dtypes.bfloat16),
                "expa": expa,
            }
        )
    return in_maps


def _host_scores(emissions, tags, mask, transitions):
    """Gold-path score: pure index gathers + sum, in float64."""
    em = np.asarray(emissions, dtype=np.float32)
    tg = np.asarray(tags).astype(np.int64)
    mk = np.asarray(mask, dtype=np.float32)
    tr = np.asarray(transitions, dtype=np.float32)

    eg = np.take_along_axis(em, tg[:, :, None], axis=2)[:, :, 0]    # (B,T)
    eg = eg * np.concatenate([np.ones((B, 1), np.float32), mk[:, 1:]], 1)
    tp = tr[tg[:, :-1], tg[:, 1:]] * mk[:, 1:]                      # (B,T-1)
    bos_t = tr[BOS, tg[:, 0]]
    last_idx = mk.astype(np.int64).sum(axis=1) - 1
    last_tags = np.take_along_axis(tg, last_idx[:, None], axis=1)[:, 0]
    eos_t = tr[last_tags, EOS]
    return (eg.astype(np.float64).sum(1) + tp.astype(np.float64).sum(1)
            + bos_t + eos_t)                                        # (B,)


# flat layouts: (column, rows) blocks aligning numerator/denominator pairs
# per boundary; see build_bass for the segment order.
_BLOCKS1 = [(0, 128), (1, 112), (2, 112), (3, 16),
            (4, 128), (5, 128), (6, 128), (7, 128), (8, 128),
            (9, 128), (10, 128), (11, 128), (12, 128), (13, 112)]
_BLOCKS2 = [(0, 128), (1, 128), (2, 96), (3, 16),
            (4, 128), (5, 128), (6, 128), (7, 128), (8, 128),
            (9, 128), (10, 128), (11, 128), (12, 128), (13, 112)]


def _host_logz(dots):
    """Fold one core's raw dot tensor into per-seq logZ (16,)."""
    d = dots.astype(np.float64)
    d1, d2 = d[:, 0:D1C], d[:, D1C : D1C + D2C]
    n1 = np.concatenate([d1[0:n, c] for c, n in _BLOCKS1])   # (NFLAT,)
    n2 = np.concatenate([d2[0:n, c] for c, n in _BLOCKS2])
    bnd = np.log(n1) - np.log(n2)
    logz = bnd.reshape(NB, BL).sum(axis=0)
    logz += np.log(d1[0:BL, D1C - 1])                        # end term
    return logz + float(T) * MU


_NC_CACHE = {}


def kernel(emissions, tags, mask, transitions):
    global LAST_RESULTS
    if "nc" not in _NC_CACHE:
        _NC_CACHE["nc"] = build_bass()
    nc = _NC_CACHE["nc"]
    in_maps = _host_prep(emissions, tags, mask, transitions)
    res = run_bass_kernel_spmd(
        nc, in_maps, core_ids=list(range(NCORES)), trace=TRACE
    )
    LAST_RESULTS = res
    scores = _host_scores(emissions, tags, mask, transitions)
    logz = np.concatenate([_host_logz(r["dots_out"]) for r in res.results])
    return np.float32(-(scores - logz).mean())


# revision 35
# speedup vs baseline: 1.1396x; 1.0167x over previous
"""CRF negative log-likelihood on 8 Trainium2 NeuronCores.

Strategy
--------
Data-parallel over batch (16 sequences per core), chunk-parallel over time
within each core. The forward recursion in the exp domain is

    u_t = exp(e_t - mu) * (M^T u_{t-1}),   M = exp(transitions)

M is a strongly mixing positive matrix (entries within 10% of 1), so a
1-step window product is numerically rank-1 and chunk chains can be
seeded with ones and stitched after the fact (rank-1 cross
approximation):

  logZ = log(end^T f_last) + sum_g [ log(1^T W_g f_{g-1}) - log(1^T W_g z) ]
         + T*mu

Schedule: emissions ship host-exp'd bf16 (DMA has headroom; Act doesn't).
Steps 1..527 form the D block: 31 chunks x 17 steps, advancing every
round (18 rounds): PE matmul -> DVE multiply straight from PSUM (1x).
Steps 528..1023 form 62 chunks x 8 steps in TWO COHORTS (X = even
chunks, Y = odd) that alternate rounds: cohort c matmuls on round r,
Act copies its PSUM to SBUF bf16 on round r+1, and the DVE multiplies it
there in 2x packed-bf16 mode. The staging chain (matmul -> copy ->
multiply) thus gets a two-round budget and stays off the critical path,
while per-round engine load is balanced:
DVE ~1000ns (496 cols 1x + 496 cols 2x), Act ~700ns (one 496-col copy),
PE ~414ns + filler matmuls that keep it continuously busy so it holds
its ramped 2.4 GHz p-state (it drops to 1.2 GHz when idling between
matmuls).

The Pool engine cannot read PSUM and its software tensor ops are ~2-4x
slower than DVE, so it only issues the streaming ee DMAs (SWDGE).

The boundary dot families come out as transposed column-sum matmuls and
are DMA'd out RAW; logs, subtraction and the per-sequence fold happen on
the host in float64. The gold-path score is host-gathered and summed on
the host (pure indexing of inputs).
"""

import json

import ml_dtypes
import numpy as np

import concourse.bass as bass
import concourse.tile as tile
import concourse.mybir as mybir
from concourse.bass_utils import run_bass_kernel_spmd
from concourse.vector_clock import ScopedClock

B, T, L = 128, 1024, 128
NCORES = 8
BL = B // NCORES          # 16 sequences per core
BOS, EOS = 126, 127
MU = float(np.log(126.0) + 0.5)

R = 18                    # global rounds
WD = 23 * BL              # D block: 15x17 + 8x16 step chunks, cols 0:368
WS = 40 * BL              # staged cols per cohort (40 chunks x 8 steps)
WTOT = WD + WS            # 1008 packed ee columns per round
NB = 102                  # chunk boundaries (22 D-internal + 80 staged)
NFLAT = NB * BL           # 1632 (boundary, seq) pairs
D1C, D2C = 15, 14         # psD1 / psD2 dot columns
D_OFFS = [1 + 17 * k for k in range(15)] + [256 + 16 * k for k in range(8)]
D_LENS = [17] * 15 + [16] * 8

FILL_PRE = 6              # prologue fillers of [L, 512]
FILL_LOOP = [512, 256]    # filler widths per round (p-state / DVFS pinning)

# ee DMA granules (round ranges); g0 on sync, the rest on gpsimd SWDGE
GRAN = [(0, 1), (1, 2), (2, 3), (3, 5), (5, 8), (8, 11), (11, 14), (14, 18)]

F32 = mybir.dt.float32
BF16 = mybir.dt.bfloat16
AF = mybir.ActivationFunctionType

TRACE = False             # set by test.py to capture an NTFF profile
LAST_RESULTS = None


# --------------------------------------------------------------------------
# Workaround for this walrus build: a Drain may carry at most ONE sync wait.
# Tile's tail drain waits on every outstanding DMA sem lane; split the waits
# across a chain of single-wait drains.
def _patch_tile_drain():
    if getattr(tile.TileContext, "_crf_drain_patched", False):
        return

    def _drain_and_barrier_split(self, tick_clock, wait_clock):
        nc = self.nc
        drain_inst = nc.sync.drain()
        wait_clock.add_sem_waits(
            drain_inst.ins, ScopedClock({None: tick_clock.global_clock})
        )
        si = drain_inst.ins.sync_info
        if si is not None and len(si.on_wait) > 1:
            waits = list(si.on_wait)
            drain_inst.ins.sync_info = mybir.SyncInfo(
                on_wait=[waits[0]], on_update=list(si.on_update)
            )
            for w in waits[1:]:
                d2 = nc.sync.drain()
                d2.ins.sync_info = mybir.SyncInfo(on_wait=[w], on_update=[])
        nc.all_engine_barrier()
        assert self.sems is not None
        popped = nc._tile_sem_poison_stack.pop()
        assert popped is self._sem_poison
        # The sem-clear ceremony (~6us of serial EVENT_SEMAPHORE traffic +
        # a second barrier) is skipped: the NEFF runs once per load and the
        # runtime reinitializes semaphore state on each execution.
        nc.free_semaphores_without_clearing(
            list(self.sems.allocated().values())
        ) if hasattr(nc, "free_semaphores_without_clearing") else None

    tile.TileContext._drain_and_barrier = _drain_and_barrier_split
    tile.TileContext._crf_drain_patched = True


# This walrus build rejects instructions carrying more than one sync wait
# ("Too many sync wait commands"). Post-process the serialized BIR: move
# excess waits onto NoOp instructions inserted just before the owner.
_MAX_WAITS = 1
_PRELOAD_WAITS = []      # (inst_name, sem_name, value) injected post-Tile
_PRELOAD_DMAS = []       # DMA inst names whose sem update must be add-imm


def _split_sync_waits_json(raw: bytes) -> bytes:
    m = json.loads(raw)
    nid = [0]
    pre = {n: (s, v) for n, s, v in _PRELOAD_WAITS}
    sem_ids = {}
    for f in m.get("functions", []):
        for s in f.get("semaphores", []):
            sem_ids[s.get("ant_name", s.get("name"))] = s.get("id")
    for f in m.get("functions", []):
        for bb in f.get("blocks", []):
            out = []
            for ins in bb.get("instructions", []):
                if ins.get("name") in _PRELOAD_DMAS:
                    for u in (ins.get("sync_info") or {}).get("on_update", []):
                        if u.get("ant_name") == "preload":
                            u["update_mode"] = "sem-add-imm"
                            u["update_value"] = 16
                if ins.get("name") in pre:
                    sname, val = pre[ins.get("name")]
                    nid[0] += 1
                    out.append({
                        "engine": ins["engine"], "ins": [],
                        "name": f"I-prewait-{nid[0]}", "opcode": "NoOp",
                        "outs": [],
                        "sync_info": {"on_update": [], "on_wait": [{
                            "ant_name": sname, "id": sem_ids.get(sname, 0),
                            "sync_type": "semaphore",
                            "wait_mode": "sem-ge-imm", "wait_value": val}]},
                    })
                si = ins.get("sync_info")
                waits = (si or {}).get("on_wait") or []
                if len(waits) > _MAX_WAITS:
                    # Keep the most-likely-critical wait on the real
                    # instruction (cross-engine compute producer, PE first);
                    # stale waits (same-engine slot reuse, DMA long done) go
                    # to the NoOps so they retire early.
                    eng = ins.get("engine", "")
                    prio = {"PE": 4, "Pool": 3, "Activation": 2}

                    def _score(w):
                        p = w.get("ant_name", "").split("_")[0]
                        if p == eng:
                            return 0
                        if p.startswith("DMA"):
                            return 1
                        return prio.get(p, 2)

                    # Same-engine sem waits are trivially satisfied on an
                    # in-order engine (no Tile loops -> no sem resets): drop.
                    waits = [
                        w
                        for w in waits
                        if w.get("ant_name", "").split("_")[0] != eng
                    ] or waits[-1:]
                    waits = sorted(waits, key=_score)
                    extra, keep = waits[:-_MAX_WAITS], waits[-_MAX_WAITS:]
                    for w in extra:
                        nid[0] += 1
                        out.append(
                            {
                                "engine": ins["engine"],
                                "ins": [],
                                "name": f"I-waitsplit-{nid[0]}",
                                "opcode": "NoOp",
                                "outs": [],
                                "sync_info": {"on_update": [], "on_wait": [w]},
                            }
                        )
                    si["on_wait"] = keep
                out.append(ins)
            bb["instructions"] = out
    return json.dumps(m).encode()


def _patch_to_json():
    if getattr(bass.Bass, "_crf_json_patched", False):
        return
    orig = bass.Bass.to_json_bytes

    def to_json_split(self, *a, **kw):
        return _split_sync_waits_json(orig(self, *a, **kw))

    bass.Bass.to_json_bytes = to_json_split
    bass.Bass._crf_json_patched = True


# --------------------------------------------------------------------------
def build_bass():
    _patch_tile_drain()
    _patch_to_json()

    nc = bass.Bass("TRN2")
    ee_d = nc.dram_tensor("ee", [L, R, WTOT], BF16, kind="ExternalInput")
    # round-0 critical data rides in two wide transfers (one completion
    # semaphore each -- DMA sems lag the data by ~2us, so fewer gates less)
    gates_d = nc.dram_tensor("gates", [L, WD + BL + L], BF16,
                             kind="ExternalInput")
    ees0_d = nc.dram_tensor("ees0", [L, WS], BF16, kind="ExternalInput")
    dots_d = nc.dram_tensor("dots_out", [L, D1C + D2C + 1], F32,
                            kind="ExternalOutput")

    with tile.TileContext(nc) as tc:
        with (
            tc.tile_pool(name="consts", bufs=1) as consts,
            tc.tile_pool(name="ud", bufs=3) as ud_pool,
            tc.tile_pool(name="ux", bufs=2) as ux_pool,
            tc.tile_pool(name="uy", bufs=2) as uy_pool,
            tc.tile_pool(name="sg", bufs=2) as sg_pool,
            tc.tile_pool(name="psd", bufs=2, space="PSUM") as psd_pool,
            tc.tile_pool(name="psx", bufs=1, space="PSUM") as psx_pool,
            tc.tile_pool(name="psy", bufs=1, space="PSUM") as psy_pool,
            tc.tile_pool(name="ps_fill", bufs=1, space="PSUM") as ps_fill,
            tc.tile_pool(name="ps_dots", bufs=1, space="PSUM") as ps_dots,
        ):
            # ---- PE warm-up fodder: no DMA dependencies ------------------
            dummy = consts.tile([L, 512], BF16)
            nc.vector.memset(dummy, 1.0)
            fillp = ps_fill.tile([L, 512], F32, name="fill", tag="fill")

            def filler(w=512):
                nc.tensor.matmul(fillp[:, 0:w], dummy[:, 0:L], dummy[:, 0:w],
                                 skip_group_check=True)

            for _ in range(FILL_PRE):
                filler()

            # ---- DMAs ----------------------------------------------------
            # gates (round-0 D ee + u0 + expa) on sync, round-0 staged ee
            # on the Act queue, the ee stream on gpsimd SWDGE.
            gates_sb = consts.tile([L, WD + BL + L], BF16)
            ees0_sb = consts.tile([L, WS], BF16)
            nc.sync.dma_start(out=gates_sb, in_=gates_d[:, :])
            nc.scalar.dma_start(out=ees0_sb, in_=ees0_d[:, :])
            expa_sb = gates_sb[:, WD + BL : WD + BL + L]
            ee_t = [(0, 1, None)]
            for gi, (lo, hi) in enumerate(GRAN):
                if gi == 0:
                    continue
                et = consts.tile([L, (hi - lo) * WTOT], BF16,
                                 name=f"ee{gi}", tag=f"ee{gi}")
                ee_t.append((lo, hi, et))
                if gi == 1:
                    nc.sync.dma_start(out=et, in_=ee_d[:, lo:hi, :])
                else:
                    nc.gpsimd.dma_start(out=et, in_=ee_d[:, lo:hi, :])
            uD = ud_pool.tile([L, WD], BF16, tag="uD")
            nc.vector.memset(uD[:, BL:WD], 1.0)
            nc.vector.tensor_copy(out=uD[:, 0:BL],
                                  in_=gates_sb[:, WD : WD + BL])
            uX = ux_pool.tile([L, WS], BF16, tag="uX")
            nc.vector.memset(uX, 1.0)
            uY = uy_pool.tile([L, WS], BF16, tag="uY")
            nc.vector.memset(uY, 1.0)

            def ee2(r, c0, c1):
                if r == 0:
                    if c1 <= WD:
                        return gates_sb[:, c0:c1]
                    assert c0 == WD and c1 == WTOT
                    return ees0_sb[:, 0:WS]
                for lo, hi, et in ee_t:
                    if lo <= r < hi:
                        base = (r - lo) * WTOT
                        return et[:, base + c0 : base + c1]
                raise AssertionError(r)

            ones_b16 = consts.tile([L, 1], BF16)
            nc.vector.memset(ones_b16, 1.0)
            endcol = consts.tile([L, 1], F32)
            nc.scalar.activation(
                out=endcol,
                in_=gates_sb[:, WD + BL + EOS : WD + BL + EOS + 1],
                func=AF.Copy)

            # transposed blocked column-sums: dots spread across
            # partitions; every matmul writes at partition 0 (HW rule), so
            # each segment starts a fresh psum column
            def blocked_colsums(psD, segs, col=0):
                for tile_, lo, hi in segs:
                    pos = lo
                    while pos < hi:
                        n = min(hi - pos, L)
                        nc.tensor.matmul(
                            psD[0:n, col : col + 1],
                            tile_[:, pos : pos + n],
                            ones_b16,
                            skip_group_check=True,
                        )
                        pos += n
                        col += 1

            # single PSUM bank: cols 0:D1C numerators (+ end term), the
            # rest denominators
            psDD = ps_dots.tile([L, D1C + D2C], F32, name="dd", tag="dd")
            psD1 = psDD[:, 0:D1C]
            psD2 = psDD[:, D1C : D1C + D2C]
            fe = consts.tile([L, BL], BF16)
            dots_sb = consts.tile([L, D1C + D2C + 1], F32)

            # ---- main scan -----------------------------------------------
            # D (23 chunks, 17/16 steps): matmul+multiply every round.
            # X (staged even chunks): matmul on even rounds, staged multiply
            # one round later; Y (odd chunks) on the odd-round phase.
            psX = psY = None
            for r in range(R):
                psD = psd_pool.tile([L, 512], F32, tag="psD")
                nc.tensor.matmul(psD[:, 0:WD], expa_sb, uD)
                # cohort matmul: 640 cols split at the PSUM bank boundary
                if r % 2 == 0:
                    psX = psx_pool.tile([L, 1024], F32, tag="psX")
                    nc.tensor.matmul(psX[:, 0:512], expa_sb, uX[:, 0:512])
                    nc.tensor.matmul(psX[:, 512:WS], expa_sb, uX[:, 512:WS])
                else:
                    psY = psy_pool.tile([L, 1024], F32, tag="psY")
                    nc.tensor.matmul(psY[:, 0:512], expa_sb, uY[:, 0:512])
                    nc.tensor.matmul(psY[:, 512:WS], expa_sb, uY[:, 512:WS])
                for w in FILL_LOOP:
                    filler(w)

                uD = ud_pool.tile([L, WD], BF16, tag="uD")
                nc.vector.tensor_mul(uD, psD[:, 0:WD], ee2(r, 0, WD))

                if r >= 1:
                    prev_is_x = (r - 1) % 2 == 0
                    pprev = psX if prev_is_x else psY
                    sg = sg_pool.tile([L, WS], BF16, tag="sg")
                    nc.scalar.activation(out=sg, in_=pprev[:, 0:WS],
                                         func=AF.Copy)
                    if prev_is_x:
                        uX = ux_pool.tile([L, WS], BF16, tag="uX")
                        nc.vector.tensor_mul(uX, sg, ee2(r - 1, WD, WTOT))
                    else:
                        uY = uy_pool.tile([L, WS], BF16, tag="uY")
                        nc.vector.tensor_mul(uY, sg, ee2(r - 1, WD, WTOT))

                if r == 0:
                    # D-internal denominators: 1^T W_k z for chunks 1..22
                    blocked_colsums(psD2, [(uD, BL, WD)], col=0)
                if r == 1:
                    # X chunks' first-step states: denominators for the
                    # D22->X0 boundary (col 3) and the Y->X boundaries
                    # (cols 9:14)
                    blocked_colsums(psD2, [(uX, 0, BL)], col=3)
                    blocked_colsums(psD2, [(uX, BL, WS)], col=9)
                if r == 2:
                    # Y chunks' first-step states: X->Y denominators
                    blocked_colsums(psD2, [(uY, 0, WS)], col=4)
                if r == 4:
                    nc.scalar.activation(out=dots_sb[:, D1C : D1C + D2C],
                                         in_=psD2, func=AF.Copy)
                if r == 16:
                    # 16-step D chunks reached their extension this round;
                    # round 17 would overwrite them with padding
                    blocked_colsums(psD1, [(uD, 240, 352)], col=2)
                    blocked_colsums(psD1, [(uD, 352, WD)], col=3)
                    # end term: chunk 79 (Y) finished its 8 own steps at the
                    # round-16 staged multiply
                    nc.vector.tensor_scalar_mul(
                        out=fe, in0=uY[:, WS - BL : WS], scalar1=endcol)
                if r == 17:
                    nc.tensor.matmul(psD1[0:BL, D1C - 1 : D1C], fe,
                                     ones_b16, skip_group_check=True)

            # ---- numerators ----------------------------------------------
            # X extension states landed with the round-17 staged multiply
            blocked_colsums(psD1, [(uX, 0, WS)], col=4)
            # Y extension: psY holds the round-17 matmul; multiply it
            # directly (1x) so the tail skips the Act staging hop
            blocked_colsums(psD1, [(uD, 0, 240)], col=0)
            uY = uy_pool.tile([L, WS], BF16, tag="uY")
            nc.vector.tensor_mul(uY, psY[:, 0:WS], ee2(R - 1, WD, WTOT))
            blocked_colsums(psD1, [(uY, 0, WS - BL)], col=9)
            nc.scalar.activation(out=dots_sb[:, 0:D1C], in_=psD1,
                                 func=AF.Copy)
            nc.sync.dma_start(out=dots_d[:, :], in_=dots_sb)

    return nc


# --------------------------------------------------------------------------
def _host_prep(emissions, tags, mask, transitions):
    em = np.asarray(emissions, dtype=np.float32)
    tr = np.asarray(transitions, dtype=np.float32)
    expa = np.exp(tr.astype(np.float64)).astype(ml_dtypes.bfloat16)

    in_maps = []
    for core in range(NCORES):
        s = slice(core * BL, (core + 1) * BL)
        emC = em[s]                                         # (BL, T, L)
        emT = np.ascontiguousarray(emC.transpose(2, 1, 0))  # (L, T, BL)

        ee = np.zeros((L, R, WTOT), np.float32)
        # D block: 23 chunks (15x17 + 8x16 steps + 1 ext round each); each
        # ext row is the successor chunk's first step (chunk 22 rolls into
        # the staged block at 384).
        for k in range(23):
            off, n = D_OFFS[k], D_LENS[k] + 1
            ee[:, 0:n, BL * k : BL * (k + 1)] = emT[:, off : off + n, :]
        # staged: 80 chunks x 8 steps, off 384+8j; X = even j on even
        # rounds (step s at round 2s), Y = odd j one round later. The ext
        # step s=8 is the successor chunk's first step; chunk 79 has no
        # successor (its ext stays 0, never read).
        for j in range(80):
            off = 384 + 8 * j
            c0 = WD + BL * (j // 2)
            par = j % 2
            for s in range(9):
                t, r = off + s, 2 * s + par
                if t < T and r < R:
                    ee[:, r, c0 : c0 + BL] = emT[:, t, :]
        ee = np.exp(ee - MU).astype(ml_dtypes.bfloat16)

        u0 = np.exp(emT[:, 0, :] + tr[BOS, :][:, None] - MU).astype(
            ml_dtypes.bfloat16)
        gates = np.concatenate([ee[:, 0, 0:WD], u0, expa], axis=1)
        in_maps.append(
            {
                "ee": ee,
                "gates": np.ascontiguousarray(gates),
                "ees0": np.ascontiguousarray(ee[:, 0, WD:WTOT]),
            }
        )
    return in_maps


def _host_scores(emissions, tags, mask, transitions):
    """Gold-path score: pure index gathers + sum, in float64."""
    em = np.asarray(emissions, dtype=np.float32)
    tg = np.asarray(tags).astype(np.int64)
    mk = np.asarray(mask, dtype=np.float32)
    tr = np.asarray(transitions, dtype=np.float32)

    eg = np.take_along_axis(em, tg[:, :, None], axis=2)[:, :, 0]    # (B,T)
    eg = eg * np.concatenate([np.ones((B, 1), np.float32), mk[:, 1:]], 1)
    tp = tr[tg[:, :-1], tg[:, 1:]] * mk[:, 1:]                      # (B,T-1)
    bos_t = tr[BOS, tg[:, 0]]
    last_idx = mk.astype(np.int64).sum(axis=1) - 1
    last_tags = np.take_along_axis(tg, last_idx[:, None], axis=1)[:, 0]
    eos_t = tr[last_tags, EOS]
    return (eg.astype(np.float64).sum(1) + tp.astype(np.float64).sum(1)
            + bos_t + eos_t)                                        # (B,)


# flat layouts: (column, rows) blocks aligning numerator/denominator pairs
# per boundary; see build_bass for the segment order.
_BLOCKS1 = [(0, 128), (1, 112), (2, 112), (3, 16),
            (4, 128), (5, 128), (6, 128), (7, 128), (8, 128),
            (9, 128), (10, 128), (11, 128), (12, 128), (13, 112)]
_BLOCKS2 = [(0, 128), (1, 128), (2, 96), (3, 16),
            (4, 128), (5, 128), (6, 128), (7, 128), (8, 128),
            (9, 128), (10, 128), (11, 128), (12, 128), (13, 112)]


def _host_logz(dots):
    """Fold one core's raw dot tensor into per-seq logZ (16,)."""
    d = dots.astype(np.float64)
    d1, d2 = d[:, 0:D1C], d[:, D1C : D1C + D2C]
    n1 = np.concatenate([d1[0:n, c] for c, n in _BLOCKS1])   # (NFLAT,)
    n2 = np.concatenate([d2[0:n, c] for c, n in _BLOCKS2])
    bnd = np.log(n1) - np.log(n2)
    logz = bnd.reshape(NB, BL).sum(axis=0)
    logz += np.log(d1[0:BL, D1C - 1])                        # end term
    return logz + float(T) * MU


_NC_CACHE = {}


def kernel(emissions, tags, mask, transitions):
    global LAST_RESULTS
    if "nc" not in _NC_CACHE:
        _NC_CACHE["nc"] = build_bass()
    nc = _NC_CACHE["nc"]
    in_maps = _host_prep(emissions, tags, mask, transitions)
    res = run_bass_kernel_spmd(
        nc, in_maps, core_ids=list(range(NCORES)), trace=TRACE
    )
    LAST_RESULTS = res
    scores = _host_scores(emissions, tags, mask, transitions)
    logz = np.concatenate([_host_logz(r["dots_out"]) for r in res.results])
    return np.float32(-(scores - logz).mean())
